# revision 4
# baseline (speedup 1.0000x reference)
"""Bidirectional LSTM (B=64, T=512, D=512, H=1024) on 8 TRN2 NeuronCores — v2.

Strategy (vs v1's 8-way gate split + per-step ncfw AllGather, ~9.7 ms):
  - 2 direction groups x 4-way tensor parallel: cores 0-3 run the forward
    direction, cores 4-7 the backward direction (bwd cores get time-reversed
    inputs, so the program is SPMD-uniform). Each core owns 256 channels of
    each gate (SH = 1024 gate columns), full batch 64.
  - Per step z = x_t @ Wx + h_{t-1} @ Wh via PE with the batch as the 64-col
    stationary and the weight shard as the 512-col moving operand. Two PSUM
    banks: bank A = (i, g) gates, bank B = (f, o) — ACT starts on A while B
    still streams.
  - h [64, 256] is PE-transposed (2 tiles), cast to bf16, and exchanged with
    the 3 same-group peers via SBUF->SBUF remote DMA (SWDGE broadcast descs,
    pre-generated one step ahead; trigger gated on the cast). Arrival is
    tracked with one semaphore per peer slot; h-matmuls of the next step are
    gated per-slot so x/own work overlaps the exchange.
  - No ncfw collective in the loop (one AllReduce barrier at startup only).
"""

import os
import numpy as np
import ml_dtypes

import concourse.bacc as bacc
import concourse.mybir as mybir
from concourse.tile import TileContext
from concourse.tile_rust import add_dep_helper
from concourse.masks import make_identity
from concourse.bass_utils import run_bass_kernel_spmd

BF16 = ml_dtypes.bfloat16

B, T, D, H = 64, 512, 512, 1024
NC = 8
GSZ = 4                 # cores per direction group
CH = H // GSZ           # hidden channels per core (256)
SH = 4 * CH             # gate shard width per core (1024)
KC_X = D // 128         # x contraction chunks (4)
N_SLOT = GSZ            # h chunk slots (own + 3 peers)
STORE_EVERY = 8

bf = mybir.dt.bfloat16
f32 = mybir.dt.float32
AF = mybir.ActivationFunctionType
ALU = mybir.AluOpType

# trn2 logical<->physical TPB map (involution), measured by v1's probe
pi = lambda x: x ^ 2 if x & 4 else x

# per remote_dma_broadcast with 8 slots: 16//8 = 2 remote sem incs at the
# one non-dummy dest, 16 local sem incs
ARR_INC_PER_SEND = 2
SEND_INC_PER_STEP = 16 * (GSZ - 1)


def build_nc(Tsteps=T, with_bias=False, Tbuf=None, exchange="remote",
             ndummy=None):
    # Tbuf: declared I/O size (>= Tsteps). Fixing Tbuf across different
    # Tsteps builds keeps host-side transfer/dispatch cost identical, so a
    # step-count sweep isolates the per-step device time.
    # exchange="none": timing-only variant — no sends/waits (results WRONG).
    Tbuf = Tsteps if Tbuf is None else Tbuf
    assert Tbuf >= Tsteps
    if ndummy is None:
        ndummy = int(os.environ.get("LSTM_DUMMY", "40"))
    NDUMMY = ndummy if exchange == "cc4" else 0
    nc = bacc.Bacc(
        None, target_bir_lowering=False, debug=False, num_devices=NC,
        # cross-core sem increments are invisible to the race detector
        detect_race_conditions=False,
    )

    xT = nc.declare_dram_parameter("xT", [128, Tbuf, KC_X, B], bf, isOutput=False)
    wh = nc.declare_dram_parameter("wh", [128, N_SLOT, 2, SH], bf, isOutput=False)
    wx = nc.declare_dram_parameter("wx", [128, KC_X, SH], bf, isOutput=False)
    if with_bias:
        bias = nc.declare_dram_parameter("bias", [1, SH], bf, isOutput=False)
    out = nc.declare_dram_parameter("out", [B, Tbuf, CH], f32, isOutput=True)

    with TileContext(nc) as tc:
        with (
            tc.tile_pool(name="const", bufs=1) as const_pool,
            tc.tile_pool(name="state", bufs=1) as state_pool,
            tc.tile_pool(name="xin", bufs=4) as x_pool,
            tc.tile_pool(name="work", bufs=2) as work_pool,
            tc.tile_pool(name="zpsA", bufs=2, space="PSUM") as psA_pool,
            tc.tile_pool(name="zpsB", bufs=2, space="PSUM") as psB_pool,
            tc.tile_pool(name="tps", bufs=2, space="PSUM") as tps_pool,
            tc.tile_pool(name="dps", bufs=1, space="PSUM") as dummy_pool,
            tc.tile_pool(name="outb", bufs=2) as out_pool,
            tc.tile_pool(name="dram", bufs=2, space="DRAM") as dram_pool,
        ):
            wh_sb = const_pool.tile([128, N_SLOT, 2, SH], bf)
            nc.sync.dma_start(out=wh_sb[:], in_=wh[:, :, :, :])
            wx_sb = const_pool.tile([128, KC_X, SH], bf)
            nc.sync.dma_start(out=wx_sb[:], in_=wx[:, :, :])
            ident = const_pool.tile([128, 128], f32)
            make_identity(nc, ident[:])
            if with_bias:
                ones_sb = const_pool.tile([1, B], bf)
                nc.gpsimd.memset(ones_sb[:], 1.0)
                bias_sb = const_pool.tile([1, SH], bf)
                nc.sync.dma_start(out=bias_sb[:], in_=bias[:, :])

            c_st = state_pool.tile([B, CH], f32)
            nc.gpsimd.memset(c_st[:], 0.0)

            # double-buffered (by step parity) exchange buffers at fixed
            # addresses: own transposed h chunk, and peer chunks by slot
            hT_own = [state_pool.tile([128, 2, B], bf, tag=f"hTown{p}",
                                      name=f"hT_own{p}")
                      for p in range(4)]
            hT_rem = [state_pool.tile([128, GSZ - 1, 2, B], bf, tag=f"hTrem{p}",
                                      name=f"hT_rem{p}")
                      for p in range(2)]
            for p in range(4):
                nc.gpsimd.memset(hT_own[p][:], 0.0)
            for p in range(2):
                nc.gpsimd.memset(hT_rem[p][:], 0.0)

            hT_all = [state_pool.tile([128, N_SLOT, 128], bf,
                                      tag=f"hTall{p}", name=f"hT_all{p}")
                      for p in range(2)]

            arrive_sems = [nc.alloc_semaphore(f"hT_arr{j}")
                           for j in range(1, GSZ)]
            send_sem = nc.alloc_semaphore("hT_send")
            clears = [nc.gpsimd.sem_clear(s)
                      for s in arrive_sems + [send_sem]]
            bar_in = dram_pool.tile([128, 1], f32, tag="barin")
            bar_out = dram_pool.tile([128, 1], f32, tag="barout")
            zr = state_pool.tile([128, 1], f32)
            nc.gpsimd.memset(zr[:], 0.0)
            nc.sync.dma_start(out=bar_in[:], in_=zr[:])
            barrier = nc.gpsimd.collective_compute(
                "AllReduce", ALU.add,
                replica_groups=[list(range(NC))],
                ins=[bar_in[:].opt()], outs=[bar_out[:].opt()],
            )
            for c in clears:
                add_dep_helper(barrier.ins, c.ins, sync=False,
                               reason="clear sems before barrier")
            prev_pool_inst = barrier

            wait_patches = []   # (ins, sem_name, value)
            out_sb = None
            h_prev = None       # h(t-1) SBUF view [64, 256] f32
            h_inst_prev = None  # DVE instruction that wrote h(t-1)
            cast_prev = None    # cast instruction of h(t-2)
            for t in range(Tsteps):
                par = t % 2
                x_sb = x_pool.tile([128, KC_X, B], bf, tag="x")
                nc.sync.dma_start(out=x_sb[:], in_=xT[:, t])

                zA = psA_pool.tile([B, 512], f32, tag="zA")
                zB = psB_pool.tile([B, 512], f32, tag="zB")

                n_mm = KC_X + (1 if with_bias else 0) + (2 * N_SLOT if t > 0 else 0)

                # --- x matmuls, both banks (no recurrence dependency) ---
                k = 0
                prev_pe = None
                for c in range(KC_X):
                    first, last = k == 0, k == n_mm - 1
                    m = nc.tensor.matmul(zA[:, :], lhsT=x_sb[:, c, :],
                                         rhs=wx_sb[:, c, 0:512],
                                         start=first, stop=last,
                                         skip_group_check=True)
                    nc.tensor.matmul(zB[:, :], lhsT=x_sb[:, c, :],
                                     rhs=wx_sb[:, c, 512:SH],
                                     start=first, stop=last,
                                     skip_group_check=True)
                    k += 1
                    prev_pe = m
                if with_bias:
                    first, last = False, k == n_mm - 1
                    nc.tensor.matmul(zA[:, :], lhsT=ones_sb[:, :],
                                     rhs=bias_sb[:, 0:512],
                                     start=first, stop=last,
                                     skip_group_check=True)
                    prev_pe = nc.tensor.matmul(zB[:, :], lhsT=ones_sb[:, :],
                                               rhs=bias_sb[:, 512:SH],
                                               start=first, stop=last,
                                               skip_group_check=True)
                    k += 1

                if t > 0:
                    par1 = (t - 1) % 2
                    # --- transpose h(t-1) -> [256ch, 64b] on PE, after the
                    # x-matmuls so PE has work while ACT/DVE finish h(t-1) ---
                    tp_ps = tps_pool.tile([128, 128], f32, tag="tp")
                    tp1 = nc.tensor.transpose(tp_ps[:, 0:B], h_prev[:, 0:128],
                                              ident[0:B, 0:B])
                    add_dep_helper(tp1.ins, prev_pe.ins, sync=False,
                                   reason="x-mms before transpose")
                    tp2 = nc.tensor.transpose(tp_ps[:, B:128], h_prev[:, 128:256],
                                              ident[0:B, 0:B])

                    # cast to bf16 into the send buffer (DVE), gated on the
                    # sends of step t-3 having drained this parity's buffer
                    if t >= 5 and exchange in ("remote", "nowait"):
                        wsend = nc.vector.wait_ge(send_sem, 0)
                        wait_patches.append(
                            (wsend.ins, "hT_send", SEND_INC_PER_STEP * (t - 4)))
                        if h_inst_prev is not None:
                            add_dep_helper(wsend.ins, h_inst_prev.ins, sync=False,
                                           reason="h update before send wait")
                    if exchange == "cc4":
                        hT_bf = work_pool.tile([128, 128], bf, tag="hTbf")
                        cast = nc.vector.tensor_copy(hT_bf[:], tp_ps[:])
                        cc_in = dram_pool.tile([128, 128], bf, tag="ccin")
                        cc_out = dram_pool.tile([N_SLOT * 128, 128], bf,
                                                tag="ccout")
                        nc.sync.dma_start(out=cc_in[:], in_=hT_bf[:])
                        nc.gpsimd.collective_compute(
                            "AllGather", ALU.bypass,
                            replica_groups=[[0, 1, 2, 3], [4, 5, 6, 7]],
                            ins=[cc_in[:].opt()], outs=[cc_out[:].opt()],
                        )
                        nc.sync.dma_start(
                            out=hT_all[par1][:],
                            in_=cc_out[:].rearrange("(j p) c -> p j c", p=128),
                        )
                    else:
                        cast = nc.vector.tensor_copy(
                            hT_own[(t - 1) % 4][:].rearrange("p c b -> p (c b)"),
                            tp_ps[:])
                    if t >= 5 and exchange in ("remote", "nowait"):
                        add_dep_helper(cast.ins, wsend.ins, sync=False,
                                       reason="cast after send-drain wait")

                    if exchange in ("remote", "nowait"):
                        # fire the (pre-generated) sends of h(t-1)
                        trig = nc.gpsimd.trigger_dma(count=None)
                        add_dep_helper(trig.ins, prev_pool_inst.ins, sync=False,
                                       reason="swdge ring order")
                        add_dep_helper(trig.ins, cast.ins, sync=True,
                                       reason="send after cast lands")
                        prev_pool_inst = trig

                    # --- keep the PE busy (HAM warm) while the exchange is
                    # in flight: throwaway matmuls into a scratch bank, in
                    # queue order between the transposes and the gated
                    # h-matmuls. Without these the PE idles >3.4us per step
                    # and HAM halves its clock for the next step's matmuls.
                    if NDUMMY and t > 0:
                        dps = dummy_pool.tile([B, 512], f32, tag="dummy")
                        prev_d = tp2
                        for _ in range(NDUMMY):
                            md = nc.tensor.matmul(
                                dps[:, :], lhsT=x_sb[:, 0, :],
                                rhs=wx_sb[:, 0, 0:512],
                                start=True, stop=True, skip_group_check=True)
                            add_dep_helper(md.ins, prev_d.ins, sync=False,
                                           reason="dummy fill order")
                            prev_d = md
                        tp2 = prev_d

                    # --- h matmuls: own slot first, then peers by slot as
                    # they arrive ---
                    prev_a = prev_b = tp2
                    for j in range(N_SLOT):
                        wj = None
                        if exchange == "cc4":
                            lhs = hT_all[par1][:, j].rearrange(
                                "p (q b) -> p q b", q=2)
                        elif j == 0 or exchange != "remote":
                            lhs = (hT_own[(t - 1) % 4] if j == 0
                                   else hT_rem[par1][:, j - 1])
                        else:
                            wj = nc.tensor.wait_ge(arrive_sems[j - 1], 0)
                            wait_patches.append(
                                (wj.ins, f"hT_arr{j}", ARR_INC_PER_SEND * t))
                            add_dep_helper(wj.ins, prev_a.ins, sync=False,
                                           reason="pe work before arrive wait")
                            add_dep_helper(wj.ins, prev_b.ins, sync=False,
                                           reason="pe work before arrive wait")
                            lhs = hT_rem[par1][:, j - 1]
                        for q in range(2):
                            first, last = False, k == n_mm - 1
                            ma = nc.tensor.matmul(zA[:, :], lhsT=lhs[:, q, :],
                                                  rhs=wh_sb[:, j, q, 0:512],
                                                  start=first, stop=last,
                                                  skip_group_check=True)
                            mb = nc.tensor.matmul(zB[:, :], lhsT=lhs[:, q, :],
                                                  rhs=wh_sb[:, j, q, 512:SH],
                                                  start=first, stop=last,
                                                  skip_group_check=True)
                            if wj is not None and q == 0:
                                add_dep_helper(ma.ins, wj.ins, sync=False,
                                               reason="arrive wait before h-mms")
                                add_dep_helper(mb.ins, wj.ins, sync=False,
                                               reason="arrive wait before h-mms")
                            if exchange == "cc4" and j == 0 and q == 0:
                                # keep the warm-keeping dummies ahead of the
                                # gather-gated matmuls in the PE queue
                                add_dep_helper(ma.ins, tp2.ins, sync=False,
                                               reason="dummies before h-mms")
                                add_dep_helper(mb.ins, tp2.ins, sync=False,
                                               reason="dummies before h-mms")
                            k += 1
                            prev_a, prev_b = ma, mb

                assert k == n_mm

                # --- gates ---
                # bank A: [i | g], bank B: [f | o]
                i_sb = work_pool.tile([B, CH], f32, tag="i")
                nc.scalar.activation(i_sb[:], zA[:, 0:CH], AF.Sigmoid)
                g_sb = work_pool.tile([B, CH], f32, tag="g")
                nc.scalar.activation(g_sb[:], zA[:, CH:512], AF.Tanh)
                ig = work_pool.tile([B, CH], f32, tag="ig")
                nc.vector.tensor_tensor(ig[:], i_sb[:], g_sb[:], ALU.mult)
                fo = work_pool.tile([B, 512], f32, tag="fo")
                nc.scalar.activation(fo[:], zB[:, :], AF.Sigmoid)
                nc.vector.tensor_tensor(c_st[:], fo[:, 0:CH], c_st[:], ALU.mult)
                nc.vector.tensor_tensor(c_st[:], c_st[:], ig[:], ALU.add)
                tc_sb = work_pool.tile([B, CH], f32, tag="tc")
                nc.scalar.activation(tc_sb[:], c_st[:], AF.Tanh)

                if t % STORE_EVERY == 0:
                    out_sb = out_pool.tile([B, STORE_EVERY, CH], f32, tag="osb")
                hview = out_sb[:, t % STORE_EVERY, :]
                h_inst_prev = nc.vector.tensor_tensor(
                    hview, fo[:, CH:512], tc_sb[:], ALU.mult)
                h_prev = out_sb[:, t % STORE_EVERY, :]

                if t % STORE_EVERY == STORE_EVERY - 1 or t == Tsteps - 1:
                    t0 = (t // STORE_EVERY) * STORE_EVERY
                    n = t + 1 - t0
                    nc.sync.dma_start(out=out[:, t0:t + 1, :],
                                      in_=out_sb[:, 0:n, :])

                # --- pre-generate the send descriptors for h(t) now, so the
                # Q7 desc-gen (~1us each) runs during the next step's matmuls
                # and the trigger only has to ring the doorbell ---
                if t < Tsteps - 1 and exchange in ("remote", "nowait"):
                    for j in range(1, GSZ):
                        rdests = [None] * 8
                        rdests[j] = (0, j)
                        prep = nc.gpsimd.remote_dma_broadcast(
                            out_ap=hT_rem[par][:, j - 1].rearrange(
                                "p q b -> p (q b)"),
                            in_ap=hT_own[t % 4][:].rearrange("p q b -> p (q b)"),
                            remote_sem=arrive_sems[j - 1],
                            local_sem=send_sem,
                            rdests=rdests,
                        )
                        add_dep_helper(prep.ins, prev_pool_inst.ins, sync=False,
                                       reason="swdge ring order")
                        prev_pool_inst = prep
                if t > 0:
                    cast_prev = cast

            if exchange in ("remote", "nowait"):
                # quiesce: all remote sends fully drained before program
                # exit, so no straggler DMA/semaphore traffic lands after
                # termination
                wq = nc.gpsimd.wait_ge(send_sem, 0)
                wait_patches.append(
                    (wq.ins, "hT_send", SEND_INC_PER_STEP * (Tsteps - 1)))
                add_dep_helper(wq.ins, prev_pool_inst.ins, sync=False,
                               reason="quiesce after last trigger")

    patched = 0
    for ins, sem_name, val in wait_patches:
        waits = ins.sync_info.on_wait
        assert len(waits) >= 1 and waits[0].ant_name == sem_name, (
            f"wait lost its sem: {ins} {sem_name}"
        )
        waits[0].wait_value = val
        patched += 1
    if exchange == "remote":
        assert patched == (Tsteps - 1) * (GSZ - 1) + max(0, Tsteps - 5) + 1, (
            patched, Tsteps)

    nc.finalize()
    return nc


def shard_inputs(inputs, Wx_f, Wh_f, b_fw, Wx_b, Wh_b, b_bw, Tsteps=T,
                 exchange="remote"):
    """Build the 8 per-core input dicts (numpy, host-side)."""
    x = np.ascontiguousarray(inputs[:, :Tsteps]).astype(BF16)   # [B,Tsteps,D]
    # xT[p, t, c, b] = x[b, t, 128c+p]
    xT_f = np.ascontiguousarray(
        x.transpose(2, 1, 0).reshape(KC_X, 128, Tsteps, B).transpose(1, 2, 0, 3)
    )
    xT_b = np.ascontiguousarray(xT_f[:, ::-1])

    with_bias = bool(np.any(b_fw) or np.any(b_bw))
    in_maps = []
    for r in range(NC):
        d = r // GSZ
        g = r % GSZ
        Wx_d, Wh_d, b_d = ((Wx_f, Wh_f, b_fw) if d == 0 else (Wx_b, Wh_b, b_bw))
        # gate column order [i, g, f, o] (reference gate order is i,f,o,g)
        cols = np.concatenate([
            np.arange(0 * H + g * CH, 0 * H + (g + 1) * CH),   # i
            np.arange(3 * H + g * CH, 3 * H + (g + 1) * CH),   # g
            np.arange(1 * H + g * CH, 1 * H + (g + 1) * CH),   # f
            np.arange(2 * H + g * CH, 2 * H + (g + 1) * CH),   # o
        ])
        wx_r = np.ascontiguousarray(
            Wx_d[:, cols].astype(BF16).reshape(KC_X, 128, SH).transpose(1, 0, 2)
        )
        # wh[p, slot j, half q, :] holds Wh rows of the h-chunk owned by the
        # sender whose data lands in slot j (slot 0 = own)
        p_phys = pi(r)
        wh_r = np.empty((128, N_SLOT, 2, SH), BF16)
        for j in range(N_SLOT):
            if exchange == "cc4":
                gl = j           # AllGather slot j = group rank j
            else:
                l = r if j == 0 else pi(p_phys ^ j)
                gl = l % GSZ
            for q in range(2):
                hc = 2 * gl + q
                wh_r[:, j, q, :] = Wh_d[128 * hc:128 * (hc + 1), cols].astype(BF16)
        m = {
            "xT": xT_f if d == 0 else xT_b,
            "wx": wx_r,
            "wh": np.ascontiguousarray(wh_r),
        }
        if with_bias:
            m["bias"] = np.ascontiguousarray(b_d[cols][None].astype(BF16))
        in_maps.append(m)
    return in_maps, with_bias


_NC_CACHE = {}


EXCHANGE = os.environ.get("LSTM_EXCHANGE", "cc4")


def run(inputs, Wx_f, Wh_f, b_fw, Wx_b, Wh_b, b_bw, Tsteps=T, trace=False,
        exchange=None):
    exchange = EXCHANGE if exchange is None else exchange
    in_maps, with_bias = shard_inputs(
        inputs, Wx_f, Wh_f, b_fw, Wx_b, Wh_b, b_bw, Tsteps, exchange
    )
    key = (Tsteps, with_bias, exchange)
    if key not in _NC_CACHE:
        _NC_CACHE[key] = build_nc(Tsteps, with_bias, exchange=exchange)
    nc = _NC_CACHE[key]
    res = run_bass_kernel_spmd(
        nc, in_maps, core_ids=list(range(NC)), trace=trace,
    )
    full = np.empty((B, Tsteps, 2 * H), np.float32)
    for r in range(NC):
        d = r // GSZ
        g = r % GSZ
        o = res.results[r]["out"]                  # [B, Tsteps, CH] f32
        if d == 1:
            o = o[:, ::-1, :]
        full[:, :, d * H + g * CH:d * H + (g + 1) * CH] = o
    return full, res


def kernel(**inputs) -> np.ndarray:
    args = (
        np.asarray(inputs["inputs"], np.float32),
        np.asarray(inputs["Wx_f"], np.float32),
        np.asarray(inputs["Wh_f"], np.float32),
        np.asarray(inputs["b_fw"], np.float32),
        np.asarray(inputs["Wx_b"], np.float32),
        np.asarray(inputs["Wh_b"], np.float32),
        np.asarray(inputs["b_bw"], np.float32),
    )
    out, _ = run(*args)
    return out
